# revision 1
# baseline (speedup 1.0000x reference)
"""Trainium2 Bass kernel for nn_AggregationLayer (smooth and/or fold over 64 columns).

Math:
  probs = softmax(selection_weights, axis=1)           # [63, 2]
  s_0 = x[:, 0]
  step i (i=1..63, using probs row i-1 = (p0, p1)):
    op_and = smoothmin(s, x_i), op_or = smoothmax(s, x_i)  (sharpness 10)
    s' = p0*op_and + p1*op_or
  Using logaddexp identities this collapses to
    s' = x_i + p0*d + gamma*softplus(10*d),   d = s - x_i,  gamma = (p1-p0)/10
  and softplus(10d) = 5d + 5|d| + log1p(exp(-10|d|)), with p0 + 5*gamma = 1/2:
    u = 0.5*d + x_i
    a = |10*d|                (ACT Abs, scale=-10)
    E = exp(-a)               (ACT Exp, scale=-1)
    L = ln(E + 1)             (ACT Ln, bias=1)
    w = 0.5*a + L
    s' = gamma*w + u
  Per step: 1 tensor_tensor + 3 scalar_tensor_tensor on DVE, 3 ACT LUT ops.

Distribution: pure data parallel over rows; 8 cores x 125,952 rows (last core
zero-padded). Host pre-transposes x so each per-step column slice is
contiguous in SBUF (no strided engine access, large contiguous DMA runs).
"""

import contextlib
import ctypes
import sys
import types

import numpy as np

P = 128          # SBUF partitions
F = 246          # rows per partition per tile
T = 4            # tiles per core
N_CORES = 8
RC = P * F * T   # 125,952 rows per core
N_ROWS = 1_000_000
N_COL = 64
N_STEP = 63

_CACHE = {}
TRACE = False
LAST = {}


# ---------------------------------------------------------------- axon NTFF shim
def _ensure_ntff_hook():
    """Provide antenv.axon_hooks (NTFF profiling) if the image lacks it."""
    try:
        from antenv.axon_hooks import get_axon_ntff_profile_hook  # noqa: F401
        return
    except ImportError:
        pass

    so_path = "/opt/axon/libaxon_pjrt.so"
    try:
        lib = ctypes.CDLL(so_path)
    except OSError:
        return
    if not hasattr(lib, "axon_start_nrt_profile"):
        return
    lib.axon_start_nrt_profile.argtypes = [ctypes.POINTER(ctypes.c_int64), ctypes.c_size_t]
    lib.axon_start_nrt_profile.restype = ctypes.c_int64
    lib.axon_stop_nrt_profile.argtypes = [ctypes.c_char_p]
    lib.axon_stop_nrt_profile.restype = ctypes.c_int64

    @contextlib.contextmanager
    def _hook(output_dir, device_ids):
        import jax

        jax.devices()
        if device_ids:
            ids = (ctypes.c_int64 * len(device_ids))(*device_ids)
            rc = lib.axon_start_nrt_profile(ids, len(device_ids))
        else:
            rc = lib.axon_start_nrt_profile(None, 0)
        if rc != 0:
            raise RuntimeError(f"axon_start_nrt_profile rc={rc}")
        try:
            yield
        finally:
            n = lib.axon_stop_nrt_profile(str(output_dir).encode())
            print(f"profile: {n} file(s) written to {output_dir}", file=sys.stderr)

    mod = types.ModuleType("antenv.axon_hooks")
    mod.get_axon_ntff_profile_hook = lambda: _hook
    mod.set_axon_ntff_profile_hook = lambda h: None
    sys.modules["antenv.axon_hooks"] = mod


# ---------------------------------------------------------------- device program
def _patch_act_tables(bacc, mybir):
    """Pin exp/ln/abs to the one set containing all three.

    Bacc's act-table-load pass assigns each activation the FIRST set that
    contains its function (exp -> exp_and_others, ln -> natural_log), which
    makes exp..ln sequences reload tables on every step (~2.6us each).
    Removing those funcs from every set except natural_log_exp_and_others
    (indices preserved) forces a single resident table set.
    """
    if getattr(bacc, "_act_tables_patched", False):
        return
    import concourse.hw_specs as hw_specs

    orig = hw_specs.get_activation_tables
    AF = mybir.ActivationFunctionType
    pinned = {AF.Exp, AF.Ln, AF.Abs}

    def patched(module_arch):
        tables = dict(orig(module_arch))
        out = {}
        for name, funcs in tables.items():
            if name == "natural_log_exp_and_others":
                out[name] = funcs
            else:
                out[name] = funcs - pinned
        return out

    bacc.get_activation_tables = patched
    bacc._act_tables_patched = True


SHIFT_C = 60.0            # exp-domain shift: exp(10d - C), ln(. + e^-C) + C
VARIANT = "delta"         # "safe" (abs+3 ACT) or "shift" (2 ACT, no abs)


def _build_nc(F=F, T=T, variant=None, dma_chunk=8, skew=1, tmp_bufs=3, sp_bufs=4):
    import concourse.bacc as bacc
    import concourse.mybir as mybir
    import concourse.tile as tile
    import math

    _patch_act_tables(bacc, mybir)
    variant = variant or VARIANT

    AF = mybir.ActivationFunctionType
    OP = mybir.AluOpType
    f32 = mybir.dt.float32
    RC = P * F * T

    nc = bacc.Bacc(None)
    xT = nc.dram_tensor("xt", [N_COL, RC], f32, kind="ExternalInput")
    # consts: cols 0..62 = gamma_i, 63..125 = p0_i, 126 = -C, 127 = exp(-C)
    gD = nc.dram_tensor("g", [P, 2 * N_STEP + 2], f32, kind="ExternalInput")
    w2D = (nc.dram_tensor("w2", [P, 2 * P], f32, kind="ExternalInput")
           if variant == "pe" else None)
    outD = nc.dram_tensor("y", [RC], f32, kind="ExternalOutput")

    with tile.TileContext(nc) as tc:
        with (
            tc.tile_pool(name="xp", bufs=3) as xp,
            tc.tile_pool(name="tmp", bufs=tmp_bufs) as tmp,
            tc.tile_pool(name="sp", bufs=sp_bufs) as sp,
            tc.tile_pool(name="gp", bufs=1) as gp,
            tc.tile_pool(name="pp", bufs=4, space="PSUM") as pp,
        ):
            g_sb = gp.tile([P, 2 * N_STEP + 2], f32)
            nc.sync.dma_start(out=g_sb[:], in_=gD[:])
            if variant == "pe":
                w2_sb = gp.tile([P, 2 * P], f32)
                nc.sync.dma_start(out=w2_sb[:], in_=w2D[:])

            def gamma_ap(i):
                return g_sb[:, i - 1 : i]

            def p0_ap(i):
                return g_sb[:, N_STEP + i - 1 : N_STEP + i]

            def emit_step_safe(s_prev, xi, i):
                d = tmp.tile([P, F], f32, tag="d")
                nc.vector.tensor_sub(d[:], s_prev, xi)
                a = tmp.tile([P, F], f32, tag="a")
                nc.scalar.activation(a[:], d[:], AF.Abs, scale=-10.0)
                E = tmp.tile([P, F], f32, tag="E")
                nc.scalar.activation(E[:], a[:], AF.Exp, scale=-1.0)
                L = tmp.tile([P, F], f32, tag="L")
                nc.scalar.activation(L[:], E[:], AF.Ln, bias=1.0)
                u = tmp.tile([P, F], f32, tag="u")
                nc.vector.scalar_tensor_tensor(u[:], d[:], 0.5, xi, OP.mult, OP.add)
                w = tmp.tile([P, F], f32, tag="w")
                nc.vector.scalar_tensor_tensor(w[:], a[:], 0.5, L[:], OP.mult, OP.add)
                s_new = sp.tile([P, F], f32, tag="s")
                nc.vector.scalar_tensor_tensor(
                    s_new[:], w[:], gamma_ap(i), u[:], OP.mult, OP.add
                )
                return s_new[:]

            def emit_step_dvabs(s_prev, xi, i):
                # abs on DVE (tensor_scalar, 2x fp32) -> ACT chain is exp,ln only
                d = tmp.tile([P, F], f32, tag="d")
                nc.vector.tensor_sub(d[:], s_prev, xi)
                a = tmp.tile([P, F], f32, tag="a")
                i32 = mybir.dt.int32
                nc.vector.tensor_scalar(
                    out=a[:].bitcast(i32), in0=d[:].bitcast(i32),
                    scalar1=0x7FFFFFFF, scalar2=0, op0=OP.bitwise_and, op1=OP.bitwise_or,
                )  # a = |d| (sign bit cleared)
                E = tmp.tile([P, F], f32, tag="E")
                nc.scalar.activation(E[:], a[:], AF.Exp, scale=-10.0)
                L = tmp.tile([P, F], f32, tag="L")
                nc.scalar.activation(L[:], E[:], AF.Ln, bias=1.0)
                u = tmp.tile([P, F], f32, tag="u")
                nc.vector.scalar_tensor_tensor(u[:], d[:], 0.5, xi, OP.mult, OP.add)
                w = tmp.tile([P, F], f32, tag="w")
                nc.vector.scalar_tensor_tensor(w[:], a[:], 5.0, L[:], OP.mult, OP.add)
                s_new = sp.tile([P, F], f32, tag="s")
                nc.vector.scalar_tensor_tensor(
                    s_new[:], w[:], gamma_ap(i), u[:], OP.mult, OP.add
                )
                return s_new[:]

            from concourse.tile_rust import add_dep_helper

            delta_ops = {}  # tile id -> list of (chunk_idx, mybir inst)

            def emit_step_delta(s_prev, xi, i, abs_on_act=False, t=None):
                # state D = s - x_next; xi holds Delta_i (or x_63 on the last step)
                # D' = 0.5*D + 5*gamma*|D| + gamma*L(|D|) + Delta_i
                a = tmp.tile([P, F], f32, tag="a")
                if abs_on_act:
                    a_inst = nc.scalar.activation(a[:], s_prev, AF.Abs)  # a = |D|
                else:
                    i32 = mybir.dt.int32
                    a_inst = nc.vector.tensor_scalar(
                        out=a[:].bitcast(i32), in0=s_prev.bitcast(i32),
                        scalar1=0x7FFFFFFF, scalar2=0, op0=OP.bitwise_and, op1=OP.bitwise_or,
                    )  # a = |D|
                E = tmp.tile([P, F], f32, tag="E")
                nc.scalar.activation(E[:], a[:], AF.Exp, scale=-10.0)
                L = tmp.tile([P, F], f32, tag="L")
                nc.scalar.activation(L[:], E[:], AF.Ln, bias=1.0)
                u = tmp.tile([P, F], f32, tag="u")
                u_inst = nc.vector.scalar_tensor_tensor(
                    u[:], s_prev, 0.5, xi, OP.mult, OP.add
                )
                # The in-place x->Delta rewrite is invisible to Tile's dep
                # tracker (out aliases in0); pin the ordering explicitly.
                if t is not None and t in delta_ops:
                    for lo, hi, dinst in delta_ops[t]:
                        if i == 1 and lo == 0:
                            add_dep_helper(a_inst.ins, dinst,
                                           reason="step1 reads Delta_0 state")
                        if i < N_COL - 1 and lo <= i < hi:
                            add_dep_helper(u_inst.ins, dinst,
                                           reason="step reads Delta col")
                w = tmp.tile([P, F], f32, tag="w")
                nc.vector.scalar_tensor_tensor(w[:], a[:], 5.0, L[:], OP.mult, OP.add)
                s_new = sp.tile([P, F], f32, tag="s")
                nc.vector.scalar_tensor_tensor(
                    s_new[:], w[:], gamma_ap(i), u[:], OP.mult, OP.add
                )
                return s_new[:]

            def emit_step_shift(s_prev, xi, i):
                # sp(10d) = ln(exp(10d - C) + e^-C) + C  (exact; no abs needed)
                # s' = p0*d + x_i + gamma*sp(10d)
                d = tmp.tile([P, F], f32, tag="d")
                nc.vector.tensor_sub(d[:], s_prev, xi)
                E = tmp.tile([P, F], f32, tag="E")
                nc.scalar.activation(
                    E[:], d[:], AF.Exp, scale=10.0, bias=g_sb[:, 126:127]
                )
                L = tmp.tile([P, F], f32, tag="L")
                nc.scalar.activation(L[:], E[:], AF.Ln, bias=g_sb[:, 127:128])
                u = tmp.tile([P, F], f32, tag="u")
                nc.vector.scalar_tensor_tensor(u[:], d[:], p0_ap(i), xi, OP.mult, OP.add)
                Lp = tmp.tile([P, F], f32, tag="Lp")
                nc.vector.tensor_scalar(
                    out=Lp[:], in0=L[:], scalar1=SHIFT_C, scalar2=gamma_ap(i),
                    op0=OP.add, op1=OP.mult,
                )
                s_new = sp.tile([P, F], f32, tag="s")
                nc.vector.tensor_add(s_new[:], Lp[:], u[:])
                return s_new[:]

            def emit_step_pe(s_prev, xi, i):
                # u = 0.5*D + Delta_i on the TensorEngine (identity matmuls
                # accumulating in PSUM); DVE does a, w, s' only.
                a = tmp.tile([P, F], f32, tag="a")
                i32 = mybir.dt.int32
                nc.vector.tensor_scalar(
                    out=a[:].bitcast(i32), in0=s_prev.bitcast(i32),
                    scalar1=0x7FFFFFFF, scalar2=0, op0=OP.bitwise_and, op1=OP.bitwise_or,
                )
                E = tmp.tile([P, F], f32, tag="E")
                nc.scalar.activation(E[:], a[:], AF.Exp, scale=-10.0)
                L = tmp.tile([P, F], f32, tag="L")
                nc.scalar.activation(L[:], E[:], AF.Ln, bias=1.0)
                u_ps = pp.tile([P, F], f32, tag="ups")
                nc.tensor.matmul(u_ps[:], w2_sb[:, 0:P], s_prev, start=True, stop=False)
                nc.tensor.matmul(u_ps[:], w2_sb[:, P : 2 * P], xi, start=False, stop=True)
                w = tmp.tile([P, F], f32, tag="w")
                nc.vector.scalar_tensor_tensor(w[:], a[:], 5.0, L[:], OP.mult, OP.add)
                s_new = sp.tile([P, F], f32, tag="s")
                nc.vector.scalar_tensor_tensor(
                    s_new[:], w[:], gamma_ap(i), u_ps[:], OP.mult, OP.add
                )
                return s_new[:]

            emit_step = {"shift": emit_step_shift, "safe": emit_step_safe,
                         "dvabs": emit_step_dvabs, "delta": emit_step_delta,
                         "delta_actabs": (lambda s_, x_, i_, t=None: emit_step_delta(s_, x_, i_, True, t=t)),
                         "delta_hyb": (lambda s_, x_, i_, t=None: emit_step_delta(s_, x_, i_, (t is not None and t % 2 == 0), t=t)),
                         "pe": emit_step_pe}[variant]

            DMA_CHUNK = dma_chunk
            SKEW = skew

            def emit_out(t, s_fin):
                dst = outD[t * P * F : (t + 1) * P * F].rearrange("(p j) -> p j", p=P)
                nc.sync.dma_start(out=dst, in_=s_fin)

            for pair in range(T // 2):
                tiles = [2 * pair, 2 * pair + 1]
                xt = {}
                for t in tiles:
                    xt[t] = xp.tile([P, N_COL, F], f32, tag="xt", name=f"xt{t}")
                    base = t * P * F
                    for c in range(0, N_COL, DMA_CHUNK):
                        src = xT[
                            c : c + DMA_CHUNK, base : base + P * F
                        ].rearrange("i (p j) -> p i j", p=P)
                        nc.sync.dma_start(out=xt[t][:, c : c + DMA_CHUNK, :], in_=src)
                    if variant.startswith("delta"):
                        # in-place x -> Delta: x[:, i, :] -= x[:, i+1, :] for i<63.
                        # Writes trail all reads of each position (reads of pos p
                        # occur at elements <= p*F, the write at p*F + pipe lat).
                        delta_ops[t] = []
                        for c in range(0, N_STEP, DMA_CHUNK):
                            hi = min(c + DMA_CHUNK, N_STEP)
                            dinst = nc.vector.tensor_sub(
                                xt[t][:, c:hi, :],
                                xt[t][:, c:hi, :],
                                xt[t][:, c + 1 : hi + 1, :],
                            )
                            delta_ops[t].append((c, hi, dinst.ins))

                tA, tB = tiles
                s_ap = {t: xt[t][:, 0, :] for t in tiles}
                for i in range(1, N_STEP + SKEW + 2):
                    if i <= N_STEP:
                        s_ap[tA] = emit_step(s_ap[tA], xt[tA][:, i, :], i,
                                             **({"t": tA} if variant.startswith("delta") else {}))
                    elif i == N_STEP + 1:
                        emit_out(tA, s_ap[tA])
                    j = i - SKEW
                    if 1 <= j <= N_STEP:
                        s_ap[tB] = emit_step(s_ap[tB], xt[tB][:, j, :], j,
                                             **({"t": tB} if variant.startswith("delta") else {}))
                    elif j == N_STEP + 1:
                        emit_out(tB, s_ap[tB])

    nc.finalize()
    return nc


def _get_nc():
    if "nc" not in _CACHE:
        _CACHE["nc"] = _build_nc()
    return _CACHE["nc"]


# ---------------------------------------------------------------- host wrapper
def kernel(x: np.ndarray, selection_weights: np.ndarray) -> np.ndarray:
    _ensure_ntff_hook()
    from concourse.bass_utils import run_bass_kernel_spmd

    nc = _get_nc()

    # softmax over the (and, or) pair, in float64 for clean constants
    w64 = selection_weights.astype(np.float64)
    e = np.exp(w64 - w64.max(axis=1, keepdims=True))
    p = e / e.sum(axis=1, keepdims=True)
    gamma = ((p[:, 1] - p[:, 0]) / 10.0).astype(np.float32)  # [63]
    p0 = p[:, 0].astype(np.float32)                          # [63]
    extra = np.array([-SHIFT_C, np.exp(-SHIFT_C)], dtype=np.float32)
    gcols = np.concatenate([gamma, p0, extra]).astype(np.float32)  # [128]
    g_arr = np.ascontiguousarray(np.broadcast_to(gcols[None, :], (P, 2 * N_STEP + 2)))

    x = np.asarray(x, dtype=np.float32)
    xT = x.T  # [64, N_ROWS] view
    in_maps = []
    for k in range(N_CORES):
        sl = xT[:, k * RC : min((k + 1) * RC, N_ROWS)]
        if sl.shape[1] < RC:
            pad = np.zeros((N_COL, RC), np.float32)
            pad[:, : sl.shape[1]] = sl
            sl = pad
        else:
            sl = np.ascontiguousarray(sl)
        in_maps.append({"xt": sl, "g": g_arr})

    res = run_bass_kernel_spmd(
        nc, in_maps, list(range(N_CORES)), trace=TRACE
    )
    LAST["exec_time_ns"] = getattr(res, "exec_time_ns", None)
    LAST["profile_json"] = getattr(res, "profile_json", None)

    out = np.concatenate([res.results[k]["y"] for k in range(N_CORES)])
    return out[:N_ROWS].reshape(N_ROWS, 1)



# revision 7
# speedup vs baseline: 1.9895x; 1.9895x over previous
"""Trainium2 Bass kernel for nn_AggregationLayer (smooth and/or fold over 64 columns).

Math (exact reformulation of the reference scan):
  probs = softmax(selection_weights, axis=1)            # [63, 2]
  s_0 = x[:, 0]
  step i (i=1..63): d = s - x_i
    s' = 0.5*(s + x_i) + 5*g_i*|d| + g_i*ln(1+exp(-10|d|)),  g_i = (p1-p0)/10
  With delta state D_{i-1} = s_{i-1} - x_i and Delta_i = x_i - x_{i+1}
  (Delta_63 = x_63, so D_63 = s_63 = output), using 0.5 + 5*g = p1:
    D_i = p1_i*D + g_i*sp(-10 D) + Delta_i            [sp = softplus]
        = 0.5*D + 5*g_i*|D| + g_i*ln(1+e^(-10|D|)) + Delta_i     (same thing)

Two per-step forms, chosen per step by an error-damping analysis:
 * cheap (steps 1..K): sp(-10D) = C + ln(exp(-10D - C) + e^-C), C = 40.
   No abs. The exp/ln LUTs at large-magnitude args carry ~5e-4 absolute
   error, but an error injected at step i is damped by prod_{k>i} max(p0,p1)
   — steps <= K=52 contribute < 3e-6 total.  2 ACT + 2 DVE stt:
     X = Exp(-10 D - C); L = Ln(X + e^-C)
     u = p1_i*D + (Delta_i + C*g_i);  D' = g_i*L + u
 * precise (steps K+1..63): the baseline abs form (LUT args near 0 where
   the tables are fp32-accurate).  2 ACT + 1 ts + 3 DVE stt:
     a = |D| (bitwise); X = Exp(-10 a); L = Ln(X + 1)
     u = 0.5*D + Delta_i;  w = 5*a + L;  D' = g_i*w + u

Distribution: pure data parallel over rows; 8 cores. Per core: C=2 chains of
[128, 492] rows; Delta columns stream through SBUF in 8-column DMA blocks
(ring of 4 blocks/chain) so DMA (32 MB/core) overlaps the whole fold.
"""

import contextlib
import ctypes
import math
import sys
import types

import numpy as np

P = 128          # SBUF partitions
C = 2            # parallel chains per core
F = 984 // C     # rows per partition per chain
BLK = 8          # columns per DMA block
N_CORES = 8
RC = P * F * C   # 125,952 rows per core
N_ROWS = 1_000_000
N_COL = 64
N_STEP = 63
RING = 4         # x blocks resident per chain
SHIFT_C = 40.0
K_CHEAP = 52     # steps 1..K use the cheap shift form

_CACHE = {}
TRACE = False
LAST = {}


# ---------------------------------------------------------------- axon NTFF shim
def _ensure_ntff_hook():
    """Provide antenv.axon_hooks (NTFF profiling) if the image lacks it."""
    try:
        from antenv.axon_hooks import get_axon_ntff_profile_hook  # noqa: F401
        return
    except ImportError:
        pass

    so_path = "/opt/axon/libaxon_pjrt.so"
    try:
        lib = ctypes.CDLL(so_path)
    except OSError:
        return
    if not hasattr(lib, "axon_start_nrt_profile"):
        return
    lib.axon_start_nrt_profile.argtypes = [ctypes.POINTER(ctypes.c_int64), ctypes.c_size_t]
    lib.axon_start_nrt_profile.restype = ctypes.c_int64
    lib.axon_stop_nrt_profile.argtypes = [ctypes.c_char_p]
    lib.axon_stop_nrt_profile.restype = ctypes.c_int64

    @contextlib.contextmanager
    def _hook(output_dir, device_ids):
        import jax

        jax.devices()
        if device_ids:
            ids = (ctypes.c_int64 * len(device_ids))(*device_ids)
            rc = lib.axon_start_nrt_profile(ids, len(device_ids))
        else:
            rc = lib.axon_start_nrt_profile(None, 0)
        if rc != 0:
            raise RuntimeError(f"axon_start_nrt_profile rc={rc}")
        try:
            yield
        finally:
            n = lib.axon_stop_nrt_profile(str(output_dir).encode())
            print(f"profile: {n} file(s) written to {output_dir}", file=sys.stderr)

    mod = types.ModuleType("antenv.axon_hooks")
    mod.get_axon_ntff_profile_hook = lambda: _hook
    mod.set_axon_ntff_profile_hook = lambda h: None
    sys.modules["antenv.axon_hooks"] = mod


# ---------------------------------------------------------------- device program
def _patch_act_tables(bacc, mybir):
    """Pin exp/ln to the one set containing both, so the whole kernel uses a
    single resident activation table (no per-step table reloads)."""
    if getattr(bacc, "_act_tables_patched", False):
        return
    AF = mybir.ActivationFunctionType
    orig = bacc.get_activation_tables
    pinned = {AF.Exp, AF.Ln, AF.Abs}

    def patched(module_arch):
        tables = dict(orig(module_arch))
        out = {}
        for name, funcs in tables.items():
            if name == "natural_log_exp_and_others":
                out[name] = funcs
            else:
                out[name] = funcs - pinned
        return out

    bacc.get_activation_tables = patched
    bacc._act_tables_patched = True


def _build_nc(C=C, F=F, blk=BLK, ring=RING, k_cheap=K_CHEAP, skew=1):
    import concourse.bacc as bacc
    import concourse.mybir as mybir
    import concourse.tile as tile

    _patch_act_tables(bacc, mybir)

    AF = mybir.ActivationFunctionType
    OP = mybir.AluOpType
    f32 = mybir.dt.float32
    i32 = mybir.dt.int32
    RC = P * F * C
    n_blocks = N_COL // blk

    nc = bacc.Bacc(None)
    # xt holds the host-precomputed Delta'' columns, transposed: [64, RC]
    xT = nc.dram_tensor("xt", [N_COL, RC], f32, kind="ExternalInput")
    # consts: cols 0..62 = gamma_i, 63..125 = p1_i, 126 = -C, 127 = e^-C
    gD = nc.dram_tensor("g", [P, 2 * N_STEP + 2], f32, kind="ExternalInput")
    outD = nc.dram_tensor("y", [RC], f32, kind="ExternalOutput")

    with tile.TileContext(nc) as tc:
        with contextlib.ExitStack() as stack:
            xpools = [
                stack.enter_context(tc.tile_pool(name=f"x{c}", bufs=ring))
                for c in range(C)
            ]
            xp = stack.enter_context(tc.tile_pool(name="xp", bufs=2 * C))
            lp = stack.enter_context(tc.tile_pool(name="lp", bufs=2 * C))
            up = stack.enter_context(tc.tile_pool(name="up", bufs=2 * C))
            ap = stack.enter_context(tc.tile_pool(name="ap", bufs=2 * C))
            wp = stack.enter_context(tc.tile_pool(name="wp", bufs=2 * C))
            stp = stack.enter_context(tc.tile_pool(name="stp", bufs=3 * C))
            gp = stack.enter_context(tc.tile_pool(name="gp", bufs=1))

            g_sb = gp.tile([P, 2 * N_STEP + 2], f32)
            nc.sync.dma_start(out=g_sb[:], in_=gD[:])

            def g_ap(i):
                return g_sb[:, i - 1 : i]

            def p1_ap(i):
                return g_sb[:, N_STEP + i - 1 : N_STEP + i]

            negC_ap = lambda: g_sb[:, 2 * N_STEP : 2 * N_STEP + 1]
            eC_ap = lambda: g_sb[:, 2 * N_STEP + 1 : 2 * N_STEP + 2]

            def load_block(c, b):
                t = xpools[c].tile([P, blk, F], f32, tag="xb")
                src = xT[
                    b * blk : (b + 1) * blk, c * P * F : (c + 1) * P * F
                ].rearrange("i (p j) -> p i j", p=P)
                nc.sync.dma_start(out=t[:], in_=src)
                return t

            # prologue: fill each chain's ring, interleaved across chains
            blocks = [[None] * n_blocks for _ in range(C)]
            for b in range(ring):
                for c in range(C):
                    blocks[c][b] = load_block(c, b)

            state = [None] * C  # AP of D_{i-1} per chain

            def emit_step(c, i):
                b, j = divmod(i, blk)
                if i == 0:
                    state[c] = blocks[c][0][:, 0, :]
                    return
                xi = blocks[c][b][:, j, :]
                d_prev = state[c]
                d_new = stp.tile([P, F], f32, tag="s")
                if i <= k_cheap:
                    # cheap shift form
                    x_t = xp.tile([P, F], f32, tag="x")
                    nc.scalar.activation(
                        x_t[:], d_prev, AF.Exp, scale=-10.0, bias=negC_ap()
                    )
                    l_t = lp.tile([P, F], f32, tag="l")
                    nc.scalar.activation(l_t[:], x_t[:], AF.Ln, bias=eC_ap())
                    u_t = up.tile([P, F], f32, tag="u")
                    nc.vector.scalar_tensor_tensor(
                        u_t[:], d_prev, p1_ap(i), xi, OP.mult, OP.add
                    )
                    nc.vector.scalar_tensor_tensor(
                        d_new[:], l_t[:], g_ap(i), u_t[:], OP.mult, OP.add
                    )
                else:
                    # precise abs form
                    a_t = ap.tile([P, F], f32, tag="a")
                    nc.vector.tensor_scalar(
                        out=a_t[:].bitcast(i32), in0=d_prev.bitcast(i32),
                        scalar1=0x7FFFFFFF, scalar2=None, op0=OP.bitwise_and,
                    )
                    x_t = xp.tile([P, F], f32, tag="x")
                    nc.scalar.activation(x_t[:], a_t[:], AF.Exp, scale=-10.0)
                    l_t = lp.tile([P, F], f32, tag="l")
                    nc.scalar.activation(l_t[:], x_t[:], AF.Ln, bias=1.0)
                    u_t = up.tile([P, F], f32, tag="u")
                    nc.vector.scalar_tensor_tensor(
                        u_t[:], d_prev, 0.5, xi, OP.mult, OP.add
                    )
                    w_t = wp.tile([P, F], f32, tag="w")
                    nc.vector.scalar_tensor_tensor(
                        w_t[:], a_t[:], 5.0, l_t[:], OP.mult, OP.add
                    )
                    nc.vector.scalar_tensor_tensor(
                        d_new[:], w_t[:], g_ap(i), u_t[:], OP.mult, OP.add
                    )
                state[c] = d_new[:]
                # refill the ring when block b's last column was consumed
                if j == blk - 1 and b + ring < n_blocks:
                    blocks[c][b + ring] = load_block(c, b + ring)
                if i == N_STEP:
                    dst = outD[c * P * F : (c + 1) * P * F].rearrange(
                        "(p j) -> p j", p=P
                    )
                    nc.sync.dma_start(out=dst, in_=d_new[:])

            for i in range(N_STEP + skew * (C - 1) + 1):
                for c in range(C):
                    ic = i - skew * c
                    if 0 <= ic <= N_STEP:
                        emit_step(c, ic)

    nc.finalize()
    return nc


def _get_nc():
    if "nc" not in _CACHE:
        _CACHE["nc"] = _build_nc()
    return _CACHE["nc"]


# ---------------------------------------------------------------- host wrapper
def kernel(x: np.ndarray, selection_weights: np.ndarray) -> np.ndarray:
    _ensure_ntff_hook()
    from concourse.bass_utils import run_bass_kernel_spmd

    nc = _get_nc()

    # softmax over the (and, or) pair, in float64 for clean constants
    w64 = selection_weights.astype(np.float64)
    e = np.exp(w64 - w64.max(axis=1, keepdims=True))
    p = e / e.sum(axis=1, keepdims=True)
    gamma = (p[:, 1] - p[:, 0]) / 10.0   # [63] float64
    p1 = p[:, 1]                         # [63] float64
    extra = np.array([-SHIFT_C, math.exp(-SHIFT_C)], dtype=np.float64)
    gcols = np.concatenate([gamma, p1, extra]).astype(np.float32)  # [128]
    g_arr = np.ascontiguousarray(
        np.broadcast_to(gcols[None, :], (P, 2 * N_STEP + 2))
    )

    x = np.asarray(x, dtype=np.float32)
    # Delta'' transform in row-major layout (contiguous ops), then transpose.
    # Cheap steps (cols 1..K_CHEAP) get the +C*gamma_i shift correction.
    corr = np.zeros(N_STEP, dtype=np.float32)
    corr[:K_CHEAP] = (SHIFT_C * gamma[:K_CHEAP]).astype(np.float32)
    dx = np.empty_like(x)
    dx[:, :N_STEP] = x[:, :N_STEP] - x[:, 1:]
    dx[:, N_STEP] = x[:, N_STEP]
    dx[:, 1:] += corr[None, :]
    dT = dx.T  # [64, N_ROWS] view

    in_maps = []
    for k in range(N_CORES):
        sl = dT[:, k * RC : min((k + 1) * RC, N_ROWS)]
        if sl.shape[1] < RC:
            pad = np.zeros((N_COL, RC), np.float32)
            pad[:, : sl.shape[1]] = sl
            sl = pad
        else:
            sl = np.ascontiguousarray(sl)
        in_maps.append({"xt": sl, "g": g_arr})

    res = run_bass_kernel_spmd(
        nc, in_maps, list(range(N_CORES)), trace=TRACE
    )
    LAST["exec_time_ns"] = getattr(res, "exec_time_ns", None)
    LAST["profile_json"] = getattr(res, "profile_json", None)

    out = np.concatenate([res.results[k]["y"] for k in range(N_CORES)])
    return out[:N_ROWS].reshape(N_ROWS, 1)


# revision 8
# speedup vs baseline: 1.9904x; 1.0004x over previous
"""Trainium2 Bass kernel for nn_AggregationLayer (smooth and/or fold over 64 columns).

Math (exact reformulation of the reference scan):
  probs = softmax(selection_weights, axis=1)            # [63, 2]
  s_0 = x[:, 0]
  step i (i=1..63): d = s - x_i
    s' = 0.5*(s + x_i) + 5*g_i*|d| + g_i*ln(1+exp(-10|d|)),  g_i = (p1-p0)/10
  With delta state D_{i-1} = s_{i-1} - x_i and Delta_i = x_i - x_{i+1}
  (Delta_63 = x_63, so D_63 = s_63 = output), using 0.5 + 5*g = p1:
    D_i = p1_i*D + g_i*sp(-10 D) + Delta_i            [sp = softplus]
        = 0.5*D + 5*g_i*|D| + g_i*ln(1+e^(-10|D|)) + Delta_i     (same thing)

Two per-step forms, chosen per step by an error-damping analysis:
 * cheap (steps 1..K): sp(-10D) = C + ln(exp(-10D - C) + e^-C), C = 40.
   No abs. The exp/ln LUTs at large-magnitude args carry ~5e-4 absolute
   error, but an error injected at step i is damped by prod_{k>i} max(p0,p1)
   — steps <= K=52 contribute < 3e-6 total.  2 ACT + 2 DVE stt:
     X = Exp(-10 D - C); L = Ln(X + e^-C)
     u = p1_i*D + (Delta_i + C*g_i);  D' = g_i*L + u
 * precise (steps K+1..63): the baseline abs form (LUT args near 0 where
   the tables are fp32-accurate).  2 ACT + 1 ts + 3 DVE stt:
     a = |D| (bitwise); X = Exp(-10 a); L = Ln(X + 1)
     u = 0.5*D + Delta_i;  w = 5*a + L;  D' = g_i*w + u

Distribution: pure data parallel over rows; 8 cores. Per core: C=2 chains of
[128, 492] rows; Delta columns stream through SBUF in 8-column DMA blocks
(ring of 4 blocks/chain) so DMA (32 MB/core) overlaps the whole fold.
"""

import contextlib
import ctypes
import math
import sys
import types

import numpy as np

P = 128          # SBUF partitions
C = 2            # parallel chains per core
F = 984 // C     # rows per partition per chain
BLK = 8          # columns per DMA block
N_CORES = 8
RC = P * F * C   # 125,952 rows per core
N_ROWS = 1_000_000
N_COL = 64
N_STEP = 63
RING = 4         # x blocks resident per chain
SHIFT_C = 40.0
K_CHEAP = 52     # steps 1..K use the cheap shift form

_CACHE = {}
TRACE = False
LAST = {}


# ---------------------------------------------------------------- axon NTFF shim
def _ensure_ntff_hook():
    """Provide antenv.axon_hooks (NTFF profiling) if the image lacks it."""
    try:
        from antenv.axon_hooks import get_axon_ntff_profile_hook  # noqa: F401
        return
    except ImportError:
        pass

    so_path = "/opt/axon/libaxon_pjrt.so"
    try:
        lib = ctypes.CDLL(so_path)
    except OSError:
        return
    if not hasattr(lib, "axon_start_nrt_profile"):
        return
    lib.axon_start_nrt_profile.argtypes = [ctypes.POINTER(ctypes.c_int64), ctypes.c_size_t]
    lib.axon_start_nrt_profile.restype = ctypes.c_int64
    lib.axon_stop_nrt_profile.argtypes = [ctypes.c_char_p]
    lib.axon_stop_nrt_profile.restype = ctypes.c_int64

    @contextlib.contextmanager
    def _hook(output_dir, device_ids):
        import jax

        jax.devices()
        if device_ids:
            ids = (ctypes.c_int64 * len(device_ids))(*device_ids)
            rc = lib.axon_start_nrt_profile(ids, len(device_ids))
        else:
            rc = lib.axon_start_nrt_profile(None, 0)
        if rc != 0:
            raise RuntimeError(f"axon_start_nrt_profile rc={rc}")
        try:
            yield
        finally:
            n = lib.axon_stop_nrt_profile(str(output_dir).encode())
            print(f"profile: {n} file(s) written to {output_dir}", file=sys.stderr)

    mod = types.ModuleType("antenv.axon_hooks")
    mod.get_axon_ntff_profile_hook = lambda: _hook
    mod.set_axon_ntff_profile_hook = lambda h: None
    sys.modules["antenv.axon_hooks"] = mod


# ---------------------------------------------------------------- device program
def _patch_act_tables(bacc, mybir):
    """Pin exp/ln to the one set containing both, so the whole kernel uses a
    single resident activation table (no per-step table reloads)."""
    if getattr(bacc, "_act_tables_patched", False):
        return
    AF = mybir.ActivationFunctionType
    orig = bacc.get_activation_tables
    pinned = {AF.Exp, AF.Ln, AF.Abs}

    def patched(module_arch):
        tables = dict(orig(module_arch))
        out = {}
        for name, funcs in tables.items():
            if name == "natural_log_exp_and_others":
                out[name] = funcs
            else:
                out[name] = funcs - pinned
        return out

    bacc.get_activation_tables = patched
    bacc._act_tables_patched = True


def _build_nc(C=C, F=F, blk=BLK, ring=RING, k_cheap=K_CHEAP, skew=1):
    import concourse.bacc as bacc
    import concourse.mybir as mybir
    import concourse.tile as tile

    _patch_act_tables(bacc, mybir)

    AF = mybir.ActivationFunctionType
    OP = mybir.AluOpType
    f32 = mybir.dt.float32
    i32 = mybir.dt.int32
    RC = P * F * C
    n_blocks = N_COL // blk

    nc = bacc.Bacc(None)
    # xt holds the host-precomputed Delta'' columns, transposed: [64, RC]
    xT = nc.dram_tensor("xt", [N_COL, RC], f32, kind="ExternalInput")
    # consts: cols 0..62 = gamma_i, 63..125 = p1_i, 126 = -C, 127 = e^-C
    gD = nc.dram_tensor("g", [P, 2 * N_STEP + 2], f32, kind="ExternalInput")
    outD = nc.dram_tensor("y", [RC], f32, kind="ExternalOutput")

    with tile.TileContext(nc) as tc:
        with contextlib.ExitStack() as stack:
            xpools = [
                stack.enter_context(tc.tile_pool(name=f"x{c}", bufs=ring))
                for c in range(C)
            ]
            xp = stack.enter_context(tc.tile_pool(name="xp", bufs=2 * C))
            lp = stack.enter_context(tc.tile_pool(name="lp", bufs=2 * C))
            up = stack.enter_context(tc.tile_pool(name="up", bufs=2 * C))
            ap = stack.enter_context(tc.tile_pool(name="ap", bufs=2 * C))
            wp = stack.enter_context(tc.tile_pool(name="wp", bufs=2 * C))
            stp = stack.enter_context(tc.tile_pool(name="stp", bufs=3 * C))
            gp = stack.enter_context(tc.tile_pool(name="gp", bufs=1))

            g_sb = gp.tile([P, 2 * N_STEP + 2], f32)
            nc.sync.dma_start(out=g_sb[:], in_=gD[:])

            def g_ap(i):
                return g_sb[:, i - 1 : i]

            def p1_ap(i):
                return g_sb[:, N_STEP + i - 1 : N_STEP + i]

            negC_ap = lambda: g_sb[:, 2 * N_STEP : 2 * N_STEP + 1]
            eC_ap = lambda: g_sb[:, 2 * N_STEP + 1 : 2 * N_STEP + 2]

            def load_block(c, b):
                t = xpools[c].tile([P, blk, F], f32, tag="xb")
                src = xT[
                    b * blk : (b + 1) * blk, c * P * F : (c + 1) * P * F
                ].rearrange("i (p j) -> p i j", p=P)
                nc.sync.dma_start(out=t[:], in_=src)
                return t

            # prologue: fill each chain's ring, interleaved across chains
            blocks = [[None] * n_blocks for _ in range(C)]
            for b in range(ring):
                for c in range(C):
                    blocks[c][b] = load_block(c, b)

            state = [None] * C   # AP of D_{i-1} per chain
            pend = [None] * C     # deferred final-combine closure per chain

            def emit_phase1(c, i):
                b, j = divmod(i, blk)
                if i == 0:
                    state[c] = blocks[c][0][:, 0, :]
                    return
                xi = blocks[c][b][:, j, :]
                d_prev = state[c]
                d_new = stp.tile([P, F], f32, tag="s")
                if i <= k_cheap:
                    # cheap shift form
                    x_t = xp.tile([P, F], f32, tag="x")
                    nc.scalar.activation(
                        x_t[:], d_prev, AF.Exp, scale=-10.0, bias=negC_ap()
                    )
                    l_t = lp.tile([P, F], f32, tag="l")
                    nc.scalar.activation(l_t[:], x_t[:], AF.Ln, bias=eC_ap())
                    u_t = up.tile([P, F], f32, tag="u")
                    nc.vector.scalar_tensor_tensor(
                        u_t[:], d_prev, p1_ap(i), xi, OP.mult, OP.add
                    )

                    def fin(c=c, i=i, l_t=l_t, u_t=u_t, d_new=d_new, b=b, j=j):
                        nc.vector.scalar_tensor_tensor(
                            d_new[:], l_t[:], g_ap(i), u_t[:], OP.mult, OP.add
                        )
                        _post(c, i, d_new, b, j)
                else:
                    # precise abs form
                    a_t = ap.tile([P, F], f32, tag="a")
                    nc.vector.tensor_scalar(
                        out=a_t[:].bitcast(i32), in0=d_prev.bitcast(i32),
                        scalar1=0x7FFFFFFF, scalar2=None, op0=OP.bitwise_and,
                    )
                    x_t = xp.tile([P, F], f32, tag="x")
                    nc.scalar.activation(x_t[:], a_t[:], AF.Exp, scale=-10.0)
                    l_t = lp.tile([P, F], f32, tag="l")
                    nc.scalar.activation(l_t[:], x_t[:], AF.Ln, bias=1.0)
                    u_t = up.tile([P, F], f32, tag="u")
                    nc.vector.scalar_tensor_tensor(
                        u_t[:], d_prev, 0.5, xi, OP.mult, OP.add
                    )

                    def fin(c=c, i=i, a_t=a_t, l_t=l_t, u_t=u_t, d_new=d_new,
                            b=b, j=j):
                        w_t = wp.tile([P, F], f32, tag="w")
                        nc.vector.scalar_tensor_tensor(
                            w_t[:], a_t[:], 5.0, l_t[:], OP.mult, OP.add
                        )
                        nc.vector.scalar_tensor_tensor(
                            d_new[:], w_t[:], g_ap(i), u_t[:], OP.mult, OP.add
                        )
                        _post(c, i, d_new, b, j)

                state[c] = d_new[:]
                pend[c] = fin

            def _post(c, i, d_new, b, j):
                # refill the ring when block b's last column was consumed
                if j == blk - 1 and b + ring < n_blocks:
                    blocks[c][b + ring] = load_block(c, b + ring)
                if i == N_STEP:
                    dst = outD[c * P * F : (c + 1) * P * F].rearrange(
                        "(p j) -> p j", p=P
                    )
                    nc.sync.dma_start(out=dst, in_=d_new[:])

            for i in range(N_STEP + skew * (C - 1) + 1):
                for c in range(C):
                    ic = i - skew * c
                    if 0 <= ic <= N_STEP:
                        emit_phase1(c, ic)
                for c in range(C):
                    ic = i - skew * c
                    if 1 <= ic <= N_STEP and pend[c] is not None:
                        pend[c]()
                        pend[c] = None

    nc.finalize()
    return nc


def _get_nc():
    if "nc" not in _CACHE:
        _CACHE["nc"] = _build_nc()
    return _CACHE["nc"]


# ---------------------------------------------------------------- host wrapper
def kernel(x: np.ndarray, selection_weights: np.ndarray) -> np.ndarray:
    _ensure_ntff_hook()
    from concourse.bass_utils import run_bass_kernel_spmd

    nc = _get_nc()

    # softmax over the (and, or) pair, in float64 for clean constants
    w64 = selection_weights.astype(np.float64)
    e = np.exp(w64 - w64.max(axis=1, keepdims=True))
    p = e / e.sum(axis=1, keepdims=True)
    gamma = (p[:, 1] - p[:, 0]) / 10.0   # [63] float64
    p1 = p[:, 1]                         # [63] float64
    extra = np.array([-SHIFT_C, math.exp(-SHIFT_C)], dtype=np.float64)
    gcols = np.concatenate([gamma, p1, extra]).astype(np.float32)  # [128]
    g_arr = np.ascontiguousarray(
        np.broadcast_to(gcols[None, :], (P, 2 * N_STEP + 2))
    )

    x = np.asarray(x, dtype=np.float32)
    # Delta'' transform in row-major layout (contiguous ops), then transpose.
    # Cheap steps (cols 1..K_CHEAP) get the +C*gamma_i shift correction.
    corr = np.zeros(N_STEP, dtype=np.float32)
    corr[:K_CHEAP] = (SHIFT_C * gamma[:K_CHEAP]).astype(np.float32)
    dx = np.empty_like(x)
    dx[:, :N_STEP] = x[:, :N_STEP] - x[:, 1:]
    dx[:, N_STEP] = x[:, N_STEP]
    dx[:, 1:] += corr[None, :]
    dT = dx.T  # [64, N_ROWS] view

    in_maps = []
    for k in range(N_CORES):
        sl = dT[:, k * RC : min((k + 1) * RC, N_ROWS)]
        if sl.shape[1] < RC:
            pad = np.zeros((N_COL, RC), np.float32)
            pad[:, : sl.shape[1]] = sl
            sl = pad
        else:
            sl = np.ascontiguousarray(sl)
        in_maps.append({"xt": sl, "g": g_arr})

    res = run_bass_kernel_spmd(
        nc, in_maps, list(range(N_CORES)), trace=TRACE
    )
    LAST["exec_time_ns"] = getattr(res, "exec_time_ns", None)
    LAST["profile_json"] = getattr(res, "profile_json", None)

    out = np.concatenate([res.results[k]["y"] for k in range(N_CORES)])
    return out[:N_ROWS].reshape(N_ROWS, 1)


# revision 9
# speedup vs baseline: 2.0654x; 1.0377x over previous
"""Trainium2 Bass kernel for nn_AggregationLayer (smooth and/or fold over 64 columns).

Math (exact reformulation of the reference scan):
  probs = softmax(selection_weights, axis=1)            # [63, 2]
  s_0 = x[:, 0]
  step i (i=1..63): d = s - x_i
    s' = 0.5*(s + x_i) + 5*g_i*|d| + g_i*ln(1+exp(-10|d|)),  g_i = (p1-p0)/10
  With delta state D_{i-1} = s_{i-1} - x_i and Delta_i = x_i - x_{i+1}
  (Delta_63 = x_63, so D_63 = s_63 = output), using 0.5 + 5*g = p1:
    D_i = p1_i*D + g_i*sp(-10 D) + Delta_i            [sp = softplus]
        = 0.5*D + 5*g_i*|D| + g_i*ln(1+e^(-10|D|)) + Delta_i     (same thing)

Two per-step forms, chosen per step by an error-damping analysis:
 * cheap (steps 1..K): sp(-10D) = C + ln(exp(-10D - C) + e^-C), C = 40.
   No abs. The exp/ln LUTs at large-magnitude args carry ~5e-4 absolute
   error, but an error injected at step i is damped by prod_{k>i} max(p0,p1)
   — steps <= K=52 contribute < 3e-6 total.  2 ACT + 2 DVE stt:
     X = Exp(-10 D - C); L = Ln(X + e^-C)
     u = p1_i*D + (Delta_i + C*g_i);  D' = g_i*L + u
 * precise (steps K+1..63): the baseline abs form (LUT args near 0 where
   the tables are fp32-accurate).  2 ACT + 1 ts + 3 DVE stt:
     a = |D| (bitwise); X = Exp(-10 a); L = Ln(X + 1)
     u = 0.5*D + Delta_i;  w = 5*a + L;  D' = g_i*w + u

Distribution: pure data parallel over rows; 8 cores. Per core: C=2 chains of
[128, 492] rows; Delta columns stream through SBUF in 8-column DMA blocks
(ring of 4 blocks/chain) so DMA (32 MB/core) overlaps the whole fold.
"""

import contextlib
import ctypes
import math
import sys
import types

import numpy as np

P = 128          # SBUF partitions
C = 2            # parallel chains per core
F = 984 // C     # rows per partition per chain
BLK = 8          # columns per DMA block
N_CORES = 8
RC = P * F * C   # 125,952 rows per core
N_ROWS = 1_000_000
N_COL = 64
N_STEP = 63
RING = 4         # x blocks resident per chain
SHIFT_C = 40.0
K_CHEAP = 56     # steps 1..K use the cheap shift form

_CACHE = {}
TRACE = False
LAST = {}


# ---------------------------------------------------------------- axon NTFF shim
def _ensure_ntff_hook():
    """Provide antenv.axon_hooks (NTFF profiling) if the image lacks it."""
    try:
        from antenv.axon_hooks import get_axon_ntff_profile_hook  # noqa: F401
        return
    except ImportError:
        pass

    so_path = "/opt/axon/libaxon_pjrt.so"
    try:
        lib = ctypes.CDLL(so_path)
    except OSError:
        return
    if not hasattr(lib, "axon_start_nrt_profile"):
        return
    lib.axon_start_nrt_profile.argtypes = [ctypes.POINTER(ctypes.c_int64), ctypes.c_size_t]
    lib.axon_start_nrt_profile.restype = ctypes.c_int64
    lib.axon_stop_nrt_profile.argtypes = [ctypes.c_char_p]
    lib.axon_stop_nrt_profile.restype = ctypes.c_int64

    @contextlib.contextmanager
    def _hook(output_dir, device_ids):
        import jax

        jax.devices()
        if device_ids:
            ids = (ctypes.c_int64 * len(device_ids))(*device_ids)
            rc = lib.axon_start_nrt_profile(ids, len(device_ids))
        else:
            rc = lib.axon_start_nrt_profile(None, 0)
        if rc != 0:
            raise RuntimeError(f"axon_start_nrt_profile rc={rc}")
        try:
            yield
        finally:
            n = lib.axon_stop_nrt_profile(str(output_dir).encode())
            print(f"profile: {n} file(s) written to {output_dir}", file=sys.stderr)

    mod = types.ModuleType("antenv.axon_hooks")
    mod.get_axon_ntff_profile_hook = lambda: _hook
    mod.set_axon_ntff_profile_hook = lambda h: None
    sys.modules["antenv.axon_hooks"] = mod


# ---------------------------------------------------------------- device program
def _patch_act_tables(bacc, mybir):
    """Pin exp/ln to the one set containing both, so the whole kernel uses a
    single resident activation table (no per-step table reloads)."""
    if getattr(bacc, "_act_tables_patched", False):
        return
    AF = mybir.ActivationFunctionType
    orig = bacc.get_activation_tables
    pinned = {AF.Exp, AF.Ln, AF.Abs}

    def patched(module_arch):
        tables = dict(orig(module_arch))
        out = {}
        for name, funcs in tables.items():
            if name == "natural_log_exp_and_others":
                out[name] = funcs
            else:
                out[name] = funcs - pinned
        return out

    bacc.get_activation_tables = patched
    bacc._act_tables_patched = True


def _build_nc(C=C, F=F, blk=BLK, ring=RING, k_cheap=K_CHEAP, skew=1):
    import concourse.bacc as bacc
    import concourse.mybir as mybir
    import concourse.tile as tile

    _patch_act_tables(bacc, mybir)

    AF = mybir.ActivationFunctionType
    OP = mybir.AluOpType
    f32 = mybir.dt.float32
    i32 = mybir.dt.int32
    RC = P * F * C
    n_blocks = N_COL // blk

    nc = bacc.Bacc(None)
    # xt holds the host-precomputed Delta'' columns, transposed: [64, RC]
    xT = nc.dram_tensor("xt", [N_COL, RC], f32, kind="ExternalInput")
    # consts: cols 0..62 = gamma_i, 63..125 = p1_i, 126 = -C, 127 = e^-C
    gD = nc.dram_tensor("g", [P, 2 * N_STEP + 2], f32, kind="ExternalInput")
    outD = nc.dram_tensor("y", [RC], f32, kind="ExternalOutput")

    with tile.TileContext(nc) as tc:
        with contextlib.ExitStack() as stack:
            xpools = [
                stack.enter_context(tc.tile_pool(name=f"x{c}", bufs=ring))
                for c in range(C)
            ]
            xp = stack.enter_context(tc.tile_pool(name="xp", bufs=2 * C))
            lp = stack.enter_context(tc.tile_pool(name="lp", bufs=2 * C))
            up = stack.enter_context(tc.tile_pool(name="up", bufs=2 * C))
            ap = stack.enter_context(tc.tile_pool(name="ap", bufs=2 * C))
            wp = stack.enter_context(tc.tile_pool(name="wp", bufs=2 * C))
            stp = stack.enter_context(tc.tile_pool(name="stp", bufs=3 * C))
            gp = stack.enter_context(tc.tile_pool(name="gp", bufs=1))

            g_sb = gp.tile([P, 2 * N_STEP + 2], f32)
            nc.sync.dma_start(out=g_sb[:], in_=gD[:])

            def g_ap(i):
                return g_sb[:, i - 1 : i]

            def p1_ap(i):
                return g_sb[:, N_STEP + i - 1 : N_STEP + i]

            negC_ap = lambda: g_sb[:, 2 * N_STEP : 2 * N_STEP + 1]
            eC_ap = lambda: g_sb[:, 2 * N_STEP + 1 : 2 * N_STEP + 2]

            def load_block(c, b):
                t = xpools[c].tile([P, blk, F], f32, tag="xb")
                src = xT[
                    b * blk : (b + 1) * blk, c * P * F : (c + 1) * P * F
                ].rearrange("i (p j) -> p i j", p=P)
                nc.sync.dma_start(out=t[:], in_=src)
                return t

            # prologue: tiny 2-col pre-blocks first so compute starts
            # ~2us in, then fill each chain's ring, interleaved across chains
            pre = []
            for c in range(C):
                t = gp.tile([P, 2, F], f32, name=f"pre{c}")
                src_ = xT[0:2, c * P * F : (c + 1) * P * F].rearrange(
                    "i (p j) -> p i j", p=P
                )
                nc.sync.dma_start(out=t[:], in_=src_)
                pre.append(t)
            blocks = [[None] * n_blocks for _ in range(C)]
            for b in range(ring):
                for c in range(C):
                    blocks[c][b] = load_block(c, b)

            state = [None] * C   # AP of D_{i-1} per chain
            pend = [None] * C     # deferred final-combine closure per chain

            def emit_phase1(c, i):
                b, j = divmod(i, blk)
                if i == 0:
                    state[c] = pre[c][:, 0, :]
                    return
                xi = pre[c][:, 1, :] if i == 1 else blocks[c][b][:, j, :]
                d_prev = state[c]
                d_new = stp.tile([P, F], f32, tag="s")
                if i <= k_cheap:
                    # cheap shift form
                    x_t = xp.tile([P, F], f32, tag="x")
                    nc.scalar.activation(
                        x_t[:], d_prev, AF.Exp, scale=-10.0, bias=negC_ap()
                    )
                    l_t = lp.tile([P, F], f32, tag="l")
                    nc.scalar.activation(l_t[:], x_t[:], AF.Ln, bias=eC_ap())
                    u_t = up.tile([P, F], f32, tag="u")
                    nc.vector.scalar_tensor_tensor(
                        u_t[:], d_prev, p1_ap(i), xi, OP.mult, OP.add
                    )

                    def fin(c=c, i=i, l_t=l_t, u_t=u_t, d_new=d_new, b=b, j=j):
                        nc.vector.scalar_tensor_tensor(
                            d_new[:], l_t[:], g_ap(i), u_t[:], OP.mult, OP.add
                        )
                        _post(c, i, d_new, b, j)
                else:
                    # precise abs form
                    a_t = ap.tile([P, F], f32, tag="a")
                    nc.vector.tensor_scalar(
                        out=a_t[:].bitcast(i32), in0=d_prev.bitcast(i32),
                        scalar1=0x7FFFFFFF, scalar2=None, op0=OP.bitwise_and,
                    )
                    x_t = xp.tile([P, F], f32, tag="x")
                    nc.scalar.activation(x_t[:], a_t[:], AF.Exp, scale=-10.0)
                    l_t = lp.tile([P, F], f32, tag="l")
                    nc.scalar.activation(l_t[:], x_t[:], AF.Ln, bias=1.0)
                    u_t = up.tile([P, F], f32, tag="u")
                    nc.vector.scalar_tensor_tensor(
                        u_t[:], d_prev, 0.5, xi, OP.mult, OP.add
                    )

                    def fin(c=c, i=i, a_t=a_t, l_t=l_t, u_t=u_t, d_new=d_new,
                            b=b, j=j):
                        w_t = wp.tile([P, F], f32, tag="w")
                        nc.vector.scalar_tensor_tensor(
                            w_t[:], a_t[:], 5.0, l_t[:], OP.mult, OP.add
                        )
                        nc.vector.scalar_tensor_tensor(
                            d_new[:], w_t[:], g_ap(i), u_t[:], OP.mult, OP.add
                        )
                        _post(c, i, d_new, b, j)

                state[c] = d_new[:]
                pend[c] = fin

            def _post(c, i, d_new, b, j):
                # refill the ring when block b's last column was consumed
                if j == blk - 1 and b + ring < n_blocks:
                    blocks[c][b + ring] = load_block(c, b + ring)
                if i == N_STEP:
                    dst = outD[c * P * F : (c + 1) * P * F].rearrange(
                        "(p j) -> p j", p=P
                    )
                    nc.sync.dma_start(out=dst, in_=d_new[:])

            for i in range(N_STEP + skew * (C - 1) + 1):
                for c in range(C):
                    ic = i - skew * c
                    if 0 <= ic <= N_STEP:
                        emit_phase1(c, ic)
                for c in range(C):
                    ic = i - skew * c
                    if 1 <= ic <= N_STEP and pend[c] is not None:
                        pend[c]()
                        pend[c] = None

    nc.finalize()
    return nc


def _get_nc():
    if "nc" not in _CACHE:
        _CACHE["nc"] = _build_nc()
    return _CACHE["nc"]


# ---------------------------------------------------------------- host wrapper
def kernel(x: np.ndarray, selection_weights: np.ndarray) -> np.ndarray:
    _ensure_ntff_hook()
    from concourse.bass_utils import run_bass_kernel_spmd

    nc = _get_nc()

    # softmax over the (and, or) pair, in float64 for clean constants
    w64 = selection_weights.astype(np.float64)
    e = np.exp(w64 - w64.max(axis=1, keepdims=True))
    p = e / e.sum(axis=1, keepdims=True)
    gamma = (p[:, 1] - p[:, 0]) / 10.0   # [63] float64
    p1 = p[:, 1]                         # [63] float64
    extra = np.array([-SHIFT_C, math.exp(-SHIFT_C)], dtype=np.float64)
    gcols = np.concatenate([gamma, p1, extra]).astype(np.float32)  # [128]
    g_arr = np.ascontiguousarray(
        np.broadcast_to(gcols[None, :], (P, 2 * N_STEP + 2))
    )

    x = np.asarray(x, dtype=np.float32)
    # Delta'' transform in row-major layout (contiguous ops), then transpose.
    # Cheap steps (cols 1..K_CHEAP) get the +C*gamma_i shift correction.
    corr = np.zeros(N_STEP, dtype=np.float32)
    corr[:K_CHEAP] = (SHIFT_C * gamma[:K_CHEAP]).astype(np.float32)
    dx = np.empty_like(x)
    dx[:, :N_STEP] = x[:, :N_STEP] - x[:, 1:]
    dx[:, N_STEP] = x[:, N_STEP]
    dx[:, 1:] += corr[None, :]
    dT = dx.T  # [64, N_ROWS] view

    in_maps = []
    for k in range(N_CORES):
        sl = dT[:, k * RC : min((k + 1) * RC, N_ROWS)]
        if sl.shape[1] < RC:
            pad = np.zeros((N_COL, RC), np.float32)
            pad[:, : sl.shape[1]] = sl
            sl = pad
        else:
            sl = np.ascontiguousarray(sl)
        in_maps.append({"xt": sl, "g": g_arr})

    res = run_bass_kernel_spmd(
        nc, in_maps, list(range(N_CORES)), trace=TRACE
    )
    LAST["exec_time_ns"] = getattr(res, "exec_time_ns", None)
    LAST["profile_json"] = getattr(res, "profile_json", None)

    out = np.concatenate([res.results[k]["y"] for k in range(N_CORES)])
    return out[:N_ROWS].reshape(N_ROWS, 1)
